# revision 8
# baseline (speedup 1.0000x reference)
"""Trainium2 Bass kernel for nn_LoraLinear (B=4, S=2048, D=4096, N=8, R=16).

Math:  y = x @ (W + sum_n softmax(s)_n B_n A_n)^T + bias
Folded: with A_cat [N*R, D] and sBT = (softmax(s)_n * B_n) concat-T [N*R, D_out]:
    t  = x @ A_cat^T                      [M, N*R]      (rank projection)
    y  = x @ W^T + t @ sBT + bias

Sharding: 8-way data-parallel over the M = B*S = 8192 token rows; every core
gets the full (host-pre-transposed) weights and 1/8 of the rows.

Per-core device program (all matmuls in float32r, 1 cyc/row):
  K is split in 2 halves of 2048 so the transposed-x panel + streamed W^T
  panel fit SBUF. Half 0 writes partial y tiles to a DRAM scratch; half 1
  reads them back, adds its own partial + the LoRA term, and writes y.
  x tiles are transposed on-chip via PE transpose (identity matmul).
"""

import os
from contextlib import ExitStack

import numpy as np

import concourse.bass as bass
import concourse.bacc as bacc
import concourse.mybir as mybir
import concourse.tile as tile
from concourse.bass_utils import run_bass_kernel_spmd
from concourse.masks import make_identity

# Problem shapes (hardcoded per harness contract)
B, S, D = 4, 2048, 4096
N_LORA, R_LORA = 8, 16
RR = N_LORA * R_LORA          # 128 folded rank
NCORES = 8
M_TOT = B * S                 # 8192
M_C = M_TOT // NCORES         # 1024 rows per core
K = D                         # contraction dim
O = D                         # out features
KH = K // 2                   # 2048 per K-half
KT = KH // 128                # 16 k-tiles per half
MT = M_C // 128               # 8 m-tiles
NB = 512                      # matmul free dim (one PSUM bank fp32)
OB = O // NB                  # 8 o-blocks

F32 = mybir.dt.float32
F32R = mybir.dt.float32r

LAST_EXEC_NS = None
LAST_RUN_S = None
_CACHED = {}


def _r(ap):
    """View an AP as float32r for the PE (bit-identical 4-byte dtype)."""
    return ap.bitcast(F32R)


def _build_nc():
    nc = bacc.Bacc("TRN2", target_bir_lowering=False, debug=False)
    xs = nc.declare_dram_parameter("xs", [M_C, K], F32, isOutput=False)
    wt = nc.declare_dram_parameter("wt", [K, O], F32, isOutput=False)      # W^T
    at = nc.declare_dram_parameter("at", [K, RR], F32, isOutput=False)     # A_cat^T
    sbt = nc.declare_dram_parameter("sbt", [RR, O], F32, isOutput=False)   # (s*B)^T
    y = nc.declare_dram_parameter("y", [M_C, O], F32, isOutput=True)

    with ExitStack() as ctx:
        tc = ctx.enter_context(tile.TileContext(nc))
        const = ctx.enter_context(tc.tile_pool(name="const", bufs=1))
        ident = const.tile([128, 128], F32)
        make_identity(nc, ident)
        sbt_t = const.tile([RR, O], F32R)

        xn_pool = ctx.enter_context(tc.tile_pool(name="xn", bufs=4))
        st_pool = ctx.enter_context(tc.tile_pool(name="stg", bufs=4))
        xt_pool = ctx.enter_context(tc.tile_pool(name="xt", bufs=1))
        at_pool = ctx.enter_context(tc.tile_pool(name="atp", bufs=3))
        wt_pool = ctx.enter_context(tc.tile_pool(name="wtp", bufs=2))
        ev_pool = ctx.enter_context(tc.tile_pool(name="ev", bufs=4))
        rb_pool = ctx.enter_context(tc.tile_pool(name="rb", bufs=4))
        t_pool = ctx.enter_context(tc.tile_pool(name="tacc", bufs=1))
        tp_ps = ctx.enter_context(tc.tile_pool(name="tp_ps", bufs=2, space="PSUM"))
        tt_ps = ctx.enter_context(tc.tile_pool(name="tt_ps", bufs=1, space="PSUM"))
        yp_ps = ctx.enter_context(tc.tile_pool(name="yp_ps", bufs=4, space="PSUM"))
        yd_pool = ctx.enter_context(tc.tile_pool(name="ydram", bufs=1, space="DRAM"))

        tpart = t_pool.tile([RR, M_C], F32R, tag="tpart")     # t^T accumulator
        ypart = yd_pool.tile([M_C, O], F32, tag="ypart")      # half-0 partial y

        for c in range(OB):
            sst = st_pool.tile([128, NB], F32, tag="stg", name=f"sst{c}")
            nc.sync.dma_start(out=sst[:, :], in_=sbt[:, c * NB : (c + 1) * NB])
            nc.vector.tensor_copy(sbt_t[:, c * NB : (c + 1) * NB], sst[:, :])

        for h in range(2):
            k0 = h * KH
            # ---- load + transpose x for this K-half: xts[i] = x^T[k-tile i] ----
            xts = [
                xt_pool.tile([128, M_C], F32R, tag=f"xt{i}", bufs=1, name=f"xt{h}_{i}") for i in range(KT)
            ]
            KC = KH // 2
            for mt in range(MT):
                for kc in range(2):
                    xn = xn_pool.tile([128, KC], F32, tag="xn", name=f"xn{h}_{mt}_{kc}")
                    nc.sync.dma_start(
                        out=xn[:, :],
                        in_=xs[mt * 128 : (mt + 1) * 128,
                               k0 + kc * KC : k0 + (kc + 1) * KC],
                    )
                    for j in range(KC // 128):
                        i = kc * (KC // 128) + j
                        tp = tp_ps.tile([128, 128], F32, tag="tp", name=f"tp{h}_{mt}_{i}")
                        nc.tensor.transpose(
                            tp[:, :], xn[:, j * 128 : (j + 1) * 128], ident
                        )
                        nc.vector.tensor_copy(
                            xts[i][:, mt * 128 : (mt + 1) * 128], tp[:, :]
                        )

            # ---- rank projection t^T += A_cat^T-half.T @ x^T-half ----
            ats = []
            for i in range(KT):
                a_t = at_pool.tile([128, RR], F32R, tag=f"at{i}", bufs=1, name=f"at{h}_{i}")
                ast = st_pool.tile([128, RR], F32, tag="stg", name=f"ast{h}_{i}")
                nc.sync.dma_start(
                    out=ast[:, :], in_=at[k0 + i * 128 : k0 + (i + 1) * 128, :]
                )
                nc.vector.tensor_copy(a_t[:, :], ast[:, :])
                ats.append(a_t)
            for mb in range(M_C // NB):
                tps = tt_ps.tile([RR, NB], F32, tag="tps", name=f"tps{h}_{mb}")
                for i in range(KT):
                    nc.tensor.matmul(
                        tps[:, :],
                        ats[i][:, :],
                        xts[i][:, mb * NB : (mb + 1) * NB],
                        start=(i == 0),
                        stop=(i == KT - 1),
                    )
                if h == 0:
                    nc.vector.tensor_copy(tpart[:, mb * NB : (mb + 1) * NB], tps[:, :])
                else:
                    nc.vector.tensor_add(
                        tpart[:, mb * NB : (mb + 1) * NB],
                        tpart[:, mb * NB : (mb + 1) * NB],
                        tps[:, :],
                    )

            # ---- main: y[mt, ob] (+)= x-half @ W^T-half (+ t @ sBT in h1) ----
            for ob in range(OB):
                wts = []
                for i in range(KT):
                    w_t = wt_pool.tile([128, NB], F32R, tag=f"wt{i}", bufs=2, name=f"wt{h}_{ob}_{i}")
                    wst = st_pool.tile([128, NB], F32, tag="stg", name=f"wst{h}_{ob}_{i}")
                    nc.sync.dma_start(
                        out=wst[:, :],
                        in_=wt[k0 + i * 128 : k0 + (i + 1) * 128,
                               ob * NB : (ob + 1) * NB],
                    )
                    nc.vector.tensor_copy(w_t[:, :], wst[:, :])
                    wts.append(w_t)
                for mt in range(MT):
                    yp = yp_ps.tile([128, NB], F32, tag="yp", name=f"yp{h}_{ob}_{mt}")
                    for i in range(KT):
                        nc.tensor.matmul(
                            yp[:, :],
                            xts[i][:, mt * 128 : (mt + 1) * 128],
                            wts[i][:, :],
                            start=(i == 0),
                            stop=(h == 0 and i == KT - 1),
                        )
                    if h == 1:
                        nc.tensor.matmul(
                            yp[:, :],
                            tpart[:, mt * 128 : (mt + 1) * 128],
                            sbt_t[:, ob * NB : (ob + 1) * NB],
                            start=False,
                            stop=True,
                        )
                    ev = ev_pool.tile([128, NB], F32, tag="ev", name=f"ev{h}_{ob}_{mt}")
                    ysl = (
                        slice(mt * 128, (mt + 1) * 128),
                        slice(ob * NB, (ob + 1) * NB),
                    )
                    if h == 0:
                        nc.vector.tensor_copy(ev[:, :], yp[:, :])
                        nc.sync.dma_start(out=ypart[ysl[0], ysl[1]], in_=ev[:, :])
                    else:
                        rb = rb_pool.tile([128, NB], F32, tag="rb", bufs=3, name=f"rb{ob}_{mt}")
                        nc.sync.dma_start(out=rb[:, :], in_=ypart[ysl[0], ysl[1]])
                        nc.vector.tensor_add(ev[:, :], yp[:, :], rb[:, :])
                        nc.sync.dma_start(out=y[ysl[0], ysl[1]], in_=ev[:, :])
    nc.finalize()
    return nc


def _host_prep(x, base_weight, base_bias, lora_score, lora_A, lora_B):
    x2 = np.ascontiguousarray(np.asarray(x, dtype=np.float32).reshape(M_TOT, K))
    w = np.asarray(base_weight, dtype=np.float32)
    s = np.asarray(lora_score, dtype=np.float64)
    s = np.exp(s - s.max())
    s = (s / s.sum()).astype(np.float32)
    a = np.asarray(lora_A, dtype=np.float32).reshape(RR, K)          # [n*r, k]
    sb = np.asarray(lora_B, dtype=np.float32) * s[:, None, None]     # [n, o, r]
    # sbt[n*r, o] matching A_cat's folded rank order
    sbt = np.ascontiguousarray(
        sb.transpose(0, 2, 1).reshape(RR, O)
    )
    wt = np.ascontiguousarray(w.T)                                   # [k, o]
    at = np.ascontiguousarray(a.T)                                   # [k, n*r]
    return x2, wt, at, sbt, np.asarray(base_bias, dtype=np.float32)


def kernel(x, base_weight, base_bias, lora_score, lora_A, lora_B):
    global LAST_EXEC_NS
    x2, wt, at, sbt, bias = _host_prep(
        x, base_weight, base_bias, lora_score, lora_A, lora_B
    )
    if "nc" not in _CACHED:
        _CACHED["nc"] = _build_nc()
    nc = _CACHED["nc"]
    in_maps = [
        {
            "xs": x2[c * M_C : (c + 1) * M_C],
            "wt": wt,
            "at": at,
            "sbt": sbt,
        }
        for c in range(NCORES)
    ]
    import time as _time

    _t0 = _time.time()
    res = run_bass_kernel_spmd(nc, in_maps, list(range(NCORES)))
    global LAST_RUN_S
    LAST_RUN_S = _time.time() - _t0
    LAST_EXEC_NS = res.exec_time_ns
    yf = np.concatenate([res.results[c]["y"] for c in range(NCORES)], axis=0)
    yf = yf + bias[None, :]
    return yf.reshape(B, S, O).astype(np.float32)
